# revision 18
# baseline (speedup 1.0000x reference)
"""CompressiveMemory (infini-attention style delta-rule memory) Trainium2 kernel.

Full inputs:
  query/key/value [4,16,4096,128] f32, M [4,16,128,128] f32, z [4,16,128,1] f32
Returns (out, M_new, z_new) matching the reference:
  sigma = elu+1;  delta rule update of (M, z) with keys/values, then retrieve
  with queries against the updated memory.

Sharding: 64 (b,h) pairs split across 8 NeuronCores, 8 heads per core.
Per-(b,h) state is independent -> embarrassingly parallel, no collectives.

Per-head algorithm on a core (S=4096, D=128, chunks of 128 rows):
  update:  norm = sig_k @ z ; r = 1/(norm+eps)
           W = sig_k^T diag(r) sig_k ; G = sig_k^T V ; csum = sig_k^T 1
           (one fused PSUM-accumulated matmul per chunk:
              lhsT=sig_k, rhs=[r*sig_k | V | 1] -> psum [W | G | csum])
           M_new = M + G - W @ M ; z_new = z + csum
  retrieve: per chunk, transpose sig_q on the PE, then
           lhsT=sig_q^T, rhs=[M_new | z_new] -> psum [mem_out | norm_q]
           out = mem_out / norm_q
This avoids ever materializing sigma_k transposed or mem_pred, so the
only PE transposes are the 32 sig_q tiles per head.  Matmul operands are
rounded to bf16 (PSUM accumulation stays fp32); M_new/z_new/out are
computed and stored in fp32.
"""

import sys
import types

sys.path.insert(0, "/opt/trn_rl_repo")

import numpy as np

HP = 8          # heads per core
S = 4096
D = 128
CH = 128        # rows per chunk
NCH = S // CH   # 32 chunks
BLK = 16        # chunks per processing block
NBLK = NCH // BLK
EPS = 1e-6
N_CORES = 8

_CACHE = {}


def _install_ntff_hook():
    """The container's antenv stub lacks axon_hooks; register it so
    trace=True can produce exec_time_ns. Harmless if already present."""
    try:
        import antenv.axon_hooks  # noqa: F401
        return
    except ImportError:
        pass
    import antenv
    mod = types.ModuleType("antenv.axon_hooks")
    _h = [None]
    mod.set_axon_ntff_profile_hook = lambda h: _h.__setitem__(0, h)
    mod.get_axon_ntff_profile_hook = lambda: _h[0]
    sys.modules["antenv.axon_hooks"] = mod
    antenv.axon_hooks = mod
    try:
        from trn_agent_boot.trn_boot import _ntff_profile_via_ctypes
        mod.set_axon_ntff_profile_hook(
            _ntff_profile_via_ctypes("/opt/axon/libaxon_pjrt.so"))
    except Exception:
        pass


def build_graph():
    import concourse.bass as bass  # noqa: F401
    import concourse.tile as tile
    from concourse import bacc, mybir, masks
    from contextlib import ExitStack

    F32 = mybir.dt.float32
    BF16 = mybir.dt.bfloat16
    AF = mybir.ActivationFunctionType
    OP = mybir.AluOpType

    nc = bacc.Bacc(None)

    q_ext = nc.declare_dram_parameter("query", [HP, S, D], F32, isOutput=False)
    k_ext = nc.declare_dram_parameter("key", [HP, S, D], F32, isOutput=False)
    v_ext = nc.declare_dram_parameter("value", [HP, S, D], F32, isOutput=False)
    m_ext = nc.declare_dram_parameter("M", [HP, D, D], F32, isOutput=False)
    z_ext = nc.declare_dram_parameter("z", [HP, D, 1], F32, isOutput=False)
    o_ext = nc.declare_dram_parameter("out", [HP, S, D], F32, isOutput=True)
    mn_ext = nc.declare_dram_parameter("M_new", [HP, D, D], F32, isOutput=True)
    zn_ext = nc.declare_dram_parameter("z_new", [HP, D, 1], F32, isOutput=True)

    with tile.TileContext(nc) as tc, ExitStack() as ctx:
        P = lambda name, bufs: ctx.enter_context(tc.tile_pool(name=name, bufs=bufs))
        PP = lambda name, bufs: ctx.enter_context(
            tc.tile_pool(name=name, bufs=bufs, space="PSUM"))

        constp = P("const", 1)
        ident = constp.tile([128, 128], BF16)
        masks.make_identity(nc, ident[:])

        ztp = P("zt", 2)
        zrp = P("zrep", 2)
        znp = P("znat", 2)
        mp_ = P("msb", 2)
        kqp = P("kq", 4)
        ep = P("exp", 2)
        wp = P("wmin", 2)
        vp = P("vrelu", 2)
        sp = P("sig", 2)
        rhsp = P("rhs", 3)
        nrmp = P("nrm", 2)
        rp = P("rcp", 2)
        scrp = P("scr", 1)
        wgp = P("wg", 2)
        mgp = P("mg", 2)
        retp = P("ret", 2)
        retbp = P("retb", 2)
        sqTp = P("sqT", 2)
        rqp = P("rq", 4)
        obp = P("outb", 2)

        pwg_pool = PP("pwg", 2)
        ps1_pool = PP("ps1", 2)   # shared: WM result (f32 512B) / transpose groups
        pr_pool = PP("pr", 2)     # retrieve outputs, 2 banks each

        for h in range(HP):
            # p-major row mapping: row s = 32*p + j lives at [p, j, :].
            # The algorithm is row-mapping-agnostic (all reductions are
            # row-local or over all of s), and this layout gives each
            # partition 4KiB-contiguous HBM runs per 8-chunk block.
            kre = k_ext[h].rearrange("(p j) d -> p j d", p=CH)
            vre = v_ext[h].rearrange("(p j) d -> p j d", p=CH)
            qre = q_ext[h].rearrange("(p j) d -> p j d", p=CH)
            ore = o_ext[h].rearrange("(p j) d -> p j d", p=CH)

            # --- per-head state loads ---
            zt = ztp.tile([1, D], BF16)
            nc.gpsimd.dma_start(out=zt[:], in_=z_ext[h, :, 0].unsqueeze(0))
            zrep = zrp.tile([128, D], BF16)
            nc.gpsimd.partition_broadcast(zrep[:], zt[0:1, :])
            znat = znp.tile([128, 1], F32)
            nc.scalar.dma_start(out=znat[:], in_=z_ext[h])
            msb = mp_.tile([128, D], F32)
            nc.scalar.dma_start(out=msb[:], in_=m_ext[h])

            scr = scrp.tile([128, D], BF16)
            pwg = pwg_pool.tile([128, 257], F32)

            # --- update phase: accumulate [W | G | csum] over 32 chunks ---
            for blk in range(NBLK):
                sl = slice(BLK * blk, BLK * blk + BLK)
                # SWDGE cast-DMA: f32 HBM -> bf16 SBUF
                kb = kqp.tile([128, BLK, D], F32)
                nc.sync.dma_start(out=kb[:], in_=kre[:, sl, :])
                rhsb = rhsp.tile([128, BLK, 258], BF16)
                nc.gpsimd.dma_start(out=rhsb[:, :, 128:256], in_=vre[:, sl, :])
                nc.gpsimd.memset(rhsb[:, :, 256:257], 1.0)

                # sigma_k = max(k+1, exp(min(k,0))) = elu(k)+1
                tb = wp.tile([128, BLK, D], BF16)
                nc.scalar.activation(tb[:], kb[:], AF.Relu, scale=-1.0)
                eb = ep.tile([128, BLK, D], BF16)
                nc.scalar.activation(eb[:], tb[:], AF.Exp, scale=-1.0)
                sk = sp.tile([128, BLK, D], BF16)
                nc.vector.scalar_tensor_tensor(
                    sk[:], kb[:], 1.0, eb[:], OP.add, OP.max)

                nrmb = nrmp.tile([128, BLK], F32)
                for c in range(BLK):
                    # norm[s] = sum_d sigma_k[s,d] * z[d]   (+eps negligible)
                    nc.vector.scalar_tensor_tensor(
                        scr[:], sk[:, c, :], 1.0, zrep[:],
                        OP.mult, OP.mult, accum_out=nrmb[:, c:c + 1])
                rb = rp.tile([128, BLK], F32)
                nc.vector.reciprocal(rb[:], nrmb[:])
                nc.vector.tensor_mul(
                    rhsb[:, :, 0:128], sk[:],
                    rb[:].unsqueeze(2).to_broadcast([128, BLK, D]))
                for c in range(BLK):
                    nc.tensor.matmul(
                        pwg[:],
                        lhsT=sk[:, c, :],
                        rhs=rhsb[:, c, 0:257],
                        start=(blk == 0 and c == 0),
                        stop=(blk == NBLK - 1 and c == BLK - 1))

            # --- finalize update: M_new = M + G - W@M ; z_new = z + csum ---
            wgs = wgp.tile([128, 257], F32)
            nc.scalar.copy(wgs[:], pwg[:])
            pwm = ps1_pool.tile([128, D], F32, tag="ps1")
            nc.tensor.matmul(
                pwm[:], lhsT=wgs[:, 0:128], rhs=msb[:], start=True, stop=True)
            mg = mgp.tile([128, D], F32)
            nc.vector.tensor_add(mg[:], msb[:], wgs[:, 128:256])
            ret = retp.tile([128, 129], F32)
            nc.vector.tensor_sub(ret[:, 0:128], mg[:], pwm[:])
            nc.vector.tensor_add(ret[:, 128:129], znat[:], wgs[:, 256:257])
            nc.gpsimd.dma_start(out=mn_ext[h], in_=ret[:, 0:128])
            nc.gpsimd.dma_start(out=zn_ext[h], in_=ret[:, 128:129])
            retb = retbp.tile([128, 129], BF16)
            nc.scalar.copy(retb[:], ret[:])

            # --- retrieve phase ---
            for blk in range(NBLK):
                sl = slice(BLK * blk, BLK * blk + BLK)
                qb = kqp.tile([128, BLK, D], F32)
                nc.scalar.dma_start(out=qb[:], in_=qre[:, sl, :])
                # sigma_q = relu(q) + exp(min(q,0)), decomposed on ACT
                tb = wp.tile([128, BLK, D], BF16)
                nc.scalar.activation(tb[:], qb[:], AF.Relu, scale=-1.0)
                eb = ep.tile([128, BLK, D], BF16)
                nc.scalar.activation(eb[:], tb[:], AF.Exp, scale=-1.0)
                vb = vp.tile([128, BLK, D], BF16)
                nc.scalar.activation(vb[:], qb[:], AF.Relu)
                sq = sp.tile([128, BLK, D], BF16)
                nc.vector.tensor_add(sq[:], eb[:], vb[:])

                ob = obp.tile([128, BLK, D], F32)
                for g in range(BLK // 4):
                    # transpose 4 chunks into one psum bank, copy out en masse
                    pt = ps1_pool.tile([128, 4, 128], BF16, tag="ps1")
                    for c4 in range(4):
                        nc.tensor.transpose(
                            pt[:, c4, :], sq[:, 4 * g + c4, :], ident[:])
                    sqT = sqTp.tile([128, 4, 128], BF16)
                    nc.any.tensor_copy(sqT[:], pt[:])
                    # 4 retrieve matmuls packed into one 2-bank psum tile:
                    # chunk j of pair b at f32 offset 512*b + 129*j
                    pr = pr_pool.tile([128, 1024], F32)
                    for c4 in range(4):
                        b2, j2 = divmod(c4, 2)
                        off = 512 * b2 + 129 * j2
                        nc.tensor.matmul(
                            pr[:, off:off + 129],
                            lhsT=sqT[:, c4, :], rhs=retb[:],
                            start=True, stop=True)
                    prv = pr[:].rearrange("p (b r) -> p b r", b=2)
                    prv = prv[:, :, 0:258].rearrange("p b (j e) -> p b j e", j=2)
                    rq = rqp.tile([128, 2, 2], F32)
                    nc.vector.reciprocal(rq[:], prv[:, :, :, 128])
                    nc.vector.tensor_mul(
                        ob[:, 4 * g:4 * g + 4, :].rearrange(
                            "p (b j) d -> p b j d", b=2),
                        prv[:, :, :, 0:128],
                        rq[:].unsqueeze(3).to_broadcast([128, 2, 2, D]))
                nc.gpsimd.dma_start(out=ore[:, sl, :], in_=ob[:])

    nc.finalize()
    return nc


def _get_graph():
    if "nc" not in _CACHE:
        _install_ntff_hook()
        _CACHE["nc"] = build_graph()
    return _CACHE["nc"]


def kernel(query, key, value, M, z, trace=False):
    from concourse.bass_utils import run_bass_kernel_spmd

    query = np.asarray(query, dtype=np.float32)
    key = np.asarray(key, dtype=np.float32)
    value = np.asarray(value, dtype=np.float32)
    M = np.asarray(M, dtype=np.float32)
    z = np.asarray(z, dtype=np.float32)

    B, H, _, _ = query.shape
    BH = B * H
    qf = query.reshape(BH, S, D)
    kf = key.reshape(BH, S, D)
    vf = value.reshape(BH, S, D)
    mf = M.reshape(BH, D, D)
    zf = z.reshape(BH, D, 1)

    nc = _get_graph()
    in_maps = []
    for i in range(N_CORES):
        sl = slice(i * HP, (i + 1) * HP)
        in_maps.append({
            "query": np.ascontiguousarray(qf[sl]),
            "key": np.ascontiguousarray(kf[sl]),
            "value": np.ascontiguousarray(vf[sl]),
            "M": np.ascontiguousarray(mf[sl]),
            "z": np.ascontiguousarray(zf[sl]),
        })

    res = run_bass_kernel_spmd(nc, in_maps, list(range(N_CORES)), trace=trace)
    out = np.concatenate([res.results[i]["out"] for i in range(N_CORES)], axis=0)
    mn = np.concatenate([res.results[i]["M_new"] for i in range(N_CORES)], axis=0)
    zn = np.concatenate([res.results[i]["z_new"] for i in range(N_CORES)], axis=0)
    ret = (out.reshape(B, H, S, D), mn.reshape(B, H, D, D), zn.reshape(B, H, D, 1))
    if trace:
        return ret, res
    return ret


# revision 20
# speedup vs baseline: 1.1831x; 1.1831x over previous
"""CompressiveMemory (infini-attention style delta-rule memory) Trainium2 kernel.

Full inputs:
  query/key/value [4,16,4096,128] f32, M [4,16,128,128] f32, z [4,16,128,1] f32
Returns (out, M_new, z_new) matching the reference:
  sigma = elu+1;  delta rule update of (M, z) with keys/values, then retrieve
  with queries against the updated memory.

Sharding: 64 (b,h) pairs split across 8 NeuronCores, 8 heads per core.
Per-(b,h) state is independent -> embarrassingly parallel, no collectives.

Per-head algorithm on a core (S=4096, D=128, chunks of 128 rows):
  update:  norm = sig_k @ z ; r = 1/(norm+eps)
           W = sig_k^T diag(r) sig_k ; G = sig_k^T V ; csum = sig_k^T 1
           (one fused PSUM-accumulated matmul per chunk:
              lhsT=sig_k, rhs=[r*sig_k | V | 1] -> psum [W | G | csum])
           M_new = M + G - W @ M ; z_new = z + csum
  retrieve: per chunk, transpose sig_q on the PE, then
           lhsT=sig_q^T, rhs=[M_new | z_new] -> psum [mem_out | norm_q]
           out = mem_out / norm_q
This avoids ever materializing sigma_k transposed or mem_pred, so the
only PE transposes are the 32 sig_q tiles per head.  Matmul operands are
rounded to bf16 (PSUM accumulation stays fp32); M_new/z_new/out are
computed and stored in fp32.
"""

import sys
import types

sys.path.insert(0, "/opt/trn_rl_repo")

import numpy as np

HP = 8          # heads per core
S = 4096
D = 128
CH = 128        # rows per chunk
NCH = S // CH   # 32 chunks
BLK = 16        # chunks per processing block
NBLK = NCH // BLK
EPS = 1e-6
N_CORES = 8

_CACHE = {}


def _install_ntff_hook():
    """The container's antenv stub lacks axon_hooks; register it so
    trace=True can produce exec_time_ns. Harmless if already present."""
    try:
        import antenv.axon_hooks  # noqa: F401
        return
    except ImportError:
        pass
    import antenv
    mod = types.ModuleType("antenv.axon_hooks")
    _h = [None]
    mod.set_axon_ntff_profile_hook = lambda h: _h.__setitem__(0, h)
    mod.get_axon_ntff_profile_hook = lambda: _h[0]
    sys.modules["antenv.axon_hooks"] = mod
    antenv.axon_hooks = mod
    try:
        from trn_agent_boot.trn_boot import _ntff_profile_via_ctypes
        mod.set_axon_ntff_profile_hook(
            _ntff_profile_via_ctypes("/opt/axon/libaxon_pjrt.so"))
    except Exception:
        pass


def build_graph():
    import concourse.bass as bass  # noqa: F401
    import concourse.tile as tile
    from concourse import bacc, mybir, masks
    from contextlib import ExitStack

    F32 = mybir.dt.float32
    BF16 = mybir.dt.bfloat16
    AF = mybir.ActivationFunctionType
    OP = mybir.AluOpType

    nc = bacc.Bacc(None)

    q_ext = nc.declare_dram_parameter("query", [HP, S, D], F32, isOutput=False)
    k_ext = nc.declare_dram_parameter("key", [HP, S, D], F32, isOutput=False)
    v_ext = nc.declare_dram_parameter("value", [HP, S, D], F32, isOutput=False)
    m_ext = nc.declare_dram_parameter("M", [HP, D, D], F32, isOutput=False)
    z_ext = nc.declare_dram_parameter("z", [HP, D, 1], F32, isOutput=False)
    o_ext = nc.declare_dram_parameter("out", [HP, S, D], F32, isOutput=True)
    mn_ext = nc.declare_dram_parameter("M_new", [HP, D, D], F32, isOutput=True)
    zn_ext = nc.declare_dram_parameter("z_new", [HP, D, 1], F32, isOutput=True)

    with tile.TileContext(nc) as tc, ExitStack() as ctx:
        P = lambda name, bufs: ctx.enter_context(tc.tile_pool(name=name, bufs=bufs))
        PP = lambda name, bufs: ctx.enter_context(
            tc.tile_pool(name=name, bufs=bufs, space="PSUM"))

        constp = P("const", 1)
        ident = constp.tile([128, 128], BF16)
        masks.make_identity(nc, ident[:])

        ztp = P("zt", 2)
        zrp = P("zrep", 2)
        znp = P("znat", 2)
        mp_ = P("msb", 2)
        kqp = P("kq", 4)
        ep = P("exp", 2)
        wp = P("wmin", 2)
        vp = P("vrelu", 2)
        sp = P("sig", 2)
        rhsp = P("rhs", 3)
        nrmp = P("nrm", 2)
        rp = P("rcp", 2)
        scrp = P("scr", 1)
        wgp = P("wg", 2)
        mgp = P("mg", 2)
        retp = P("ret", 2)
        retbp = P("retb", 2)
        sqTp = P("sqT", 2)
        rqp = P("rq", 4)
        obp = P("outb", 2)

        pwg_pool = PP("pwg", 2)
        ps1_pool = PP("ps1", 2)   # shared: WM result (f32 512B) / transpose groups
        pr_pool = PP("pr", 2)     # retrieve outputs, 2 banks each

        def emit_update_start(h):
            # p-major row mapping: row s = 32*p + j lives at [p, j, :].
            # The algorithm is row-mapping-agnostic (all reductions are
            # row-local or over all of s), and this layout gives each
            # partition 4KiB-contiguous HBM runs per 8-chunk block.
            kre = k_ext[h].rearrange("(p j) d -> p j d", p=CH)
            vre = v_ext[h].rearrange("(p j) d -> p j d", p=CH)

            # --- per-head state loads ---
            zt = ztp.tile([1, D], BF16)
            nc.gpsimd.dma_start(out=zt[:], in_=z_ext[h, :, 0].unsqueeze(0))
            zrep = zrp.tile([128, D], BF16)
            nc.gpsimd.partition_broadcast(zrep[:], zt[0:1, :])
            znat = znp.tile([128, 1], F32)
            nc.scalar.dma_start(out=znat[:], in_=z_ext[h])
            msb = mp_.tile([128, D], F32)
            nc.scalar.dma_start(out=msb[:], in_=m_ext[h])

            scr = scrp.tile([128, D], BF16)
            pwg = pwg_pool.tile([128, 257], F32)
            return dict(kre=kre, vre=vre, zrep=zrep, znat=znat, msb=msb,
                        scr=scr, pwg=pwg)

        def emit_update_block(h, st, blk):
            # --- update phase: accumulate [W | G | csum] over the chunks ---
            kre, vre, zrep, scr, pwg = \
                st["kre"], st["vre"], st["zrep"], st["scr"], st["pwg"]
            if True:
                sl = slice(BLK * blk, BLK * blk + BLK)
                # SWDGE cast-DMA: f32 HBM -> bf16 SBUF
                kb = kqp.tile([128, BLK, D], F32)
                nc.sync.dma_start(out=kb[:], in_=kre[:, sl, :])
                rhsb = rhsp.tile([128, BLK, 258], BF16)
                nc.gpsimd.dma_start(out=rhsb[:, :, 128:256], in_=vre[:, sl, :])
                nc.gpsimd.memset(rhsb[:, :, 256:257], 1.0)

                # sigma_k = max(k+1, exp(min(k,0))) = elu(k)+1
                tb = wp.tile([128, BLK, D], BF16)
                nc.scalar.activation(tb[:], kb[:], AF.Relu, scale=-1.0)
                eb = ep.tile([128, BLK, D], BF16)
                nc.scalar.activation(eb[:], tb[:], AF.Exp, scale=-1.0)
                sk = sp.tile([128, BLK, D], BF16)
                nc.vector.scalar_tensor_tensor(
                    sk[:], kb[:], 1.0, eb[:], OP.add, OP.max)

                nrmb = nrmp.tile([128, BLK], F32)
                for c in range(BLK):
                    # norm[s] = sum_d sigma_k[s,d] * z[d]   (+eps negligible)
                    nc.vector.scalar_tensor_tensor(
                        scr[:], sk[:, c, :], 1.0, zrep[:],
                        OP.mult, OP.mult, accum_out=nrmb[:, c:c + 1])
                rb = rp.tile([128, BLK], F32)
                nc.vector.reciprocal(rb[:], nrmb[:])
                nc.vector.tensor_mul(
                    rhsb[:, :, 0:128], sk[:],
                    rb[:].unsqueeze(2).to_broadcast([128, BLK, D]))
                for c in range(BLK):
                    nc.tensor.matmul(
                        pwg[:],
                        lhsT=sk[:, c, :],
                        rhs=rhsb[:, c, 0:257],
                        start=(blk == 0 and c == 0),
                        stop=(blk == NBLK - 1 and c == BLK - 1))

        def emit_update_end(h, st):
            msb, znat, pwg = st["msb"], st["znat"], st["pwg"]
            # --- finalize update: M_new = M + G - W@M ; z_new = z + csum ---
            wgs = wgp.tile([128, 257], F32)
            nc.scalar.copy(wgs[:], pwg[:])
            pwm = ps1_pool.tile([128, D], F32, tag="ps1")
            nc.tensor.matmul(
                pwm[:], lhsT=wgs[:, 0:128], rhs=msb[:], start=True, stop=True)
            mg = mgp.tile([128, D], F32)
            nc.vector.tensor_add(mg[:], msb[:], wgs[:, 128:256])
            ret = retp.tile([128, 129], F32)
            nc.vector.tensor_sub(ret[:, 0:128], mg[:], pwm[:])
            nc.vector.tensor_add(ret[:, 128:129], znat[:], wgs[:, 256:257])
            nc.gpsimd.dma_start(out=mn_ext[h], in_=ret[:, 0:128])
            nc.gpsimd.dma_start(out=zn_ext[h], in_=ret[:, 128:129])
            retb = retbp.tile([128, 129], BF16)
            nc.scalar.copy(retb[:], ret[:])
            return retb

        def emit_retrieve_block(h, retb, blk):
            qre = q_ext[h].rearrange("(p j) d -> p j d", p=CH)
            ore = o_ext[h].rearrange("(p j) d -> p j d", p=CH)
            # --- retrieve phase ---
            if True:
                sl = slice(BLK * blk, BLK * blk + BLK)
                qb = kqp.tile([128, BLK, D], F32)
                nc.scalar.dma_start(out=qb[:], in_=qre[:, sl, :])
                # sigma_q = relu(q) + exp(min(q,0)), decomposed on ACT
                tb = wp.tile([128, BLK, D], BF16)
                nc.scalar.activation(tb[:], qb[:], AF.Relu, scale=-1.0)
                eb = ep.tile([128, BLK, D], BF16)
                nc.scalar.activation(eb[:], tb[:], AF.Exp, scale=-1.0)
                vb = vp.tile([128, BLK, D], BF16)
                nc.scalar.activation(vb[:], qb[:], AF.Relu)
                sq = sp.tile([128, BLK, D], BF16)
                nc.vector.tensor_add(sq[:], eb[:], vb[:])

                ob = obp.tile([128, BLK, D], F32)
                for g in range(BLK // 4):
                    # transpose 4 chunks into one psum bank, copy out en masse
                    pt = ps1_pool.tile([128, 4, 128], BF16, tag="ps1")
                    for c4 in range(4):
                        nc.tensor.transpose(
                            pt[:, c4, :], sq[:, 4 * g + c4, :], ident[:])
                    sqT = sqTp.tile([128, 4, 128], BF16)
                    nc.any.tensor_copy(sqT[:], pt[:])
                    # 4 retrieve matmuls packed into one 2-bank psum tile:
                    # chunk j of pair b at f32 offset 512*b + 129*j
                    pr = pr_pool.tile([128, 1024], F32)
                    for c4 in range(4):
                        b2, j2 = divmod(c4, 2)
                        off = 512 * b2 + 129 * j2
                        nc.tensor.matmul(
                            pr[:, off:off + 129],
                            lhsT=sqT[:, c4, :], rhs=retb[:],
                            start=True, stop=True)
                    prv = pr[:].rearrange("p (b r) -> p b r", b=2)
                    prv = prv[:, :, 0:258].rearrange("p b (j e) -> p b j e", j=2)
                    rq = rqp.tile([128, 2, 2], F32)
                    nc.vector.reciprocal(rq[:], prv[:, :, :, 128])
                    nc.vector.tensor_mul(
                        ob[:, 4 * g:4 * g + 4, :].rearrange(
                            "p (b j) d -> p b j d", b=2),
                        prv[:, :, :, 0:128],
                        rq[:].unsqueeze(3).to_broadcast([128, 2, 2, D]))
                nc.gpsimd.dma_start(out=ore[:, sl, :], in_=ob[:])

        # software-pipelined schedule: retrieve(h-1) interleaves update(h)
        prev = None
        for h in range(HP):
            st = emit_update_start(h)
            for blk in range(NBLK):
                emit_update_block(h, st, blk)
                if prev is not None:
                    emit_retrieve_block(prev[0], prev[1], blk)
            prev = (h, emit_update_end(h, st))
        for blk in range(NBLK):
            emit_retrieve_block(prev[0], prev[1], blk)

    nc.finalize()
    return nc


def _get_graph():
    if "nc" not in _CACHE:
        _install_ntff_hook()
        _CACHE["nc"] = build_graph()
    return _CACHE["nc"]


def kernel(query, key, value, M, z, trace=False):
    from concourse.bass_utils import run_bass_kernel_spmd

    query = np.asarray(query, dtype=np.float32)
    key = np.asarray(key, dtype=np.float32)
    value = np.asarray(value, dtype=np.float32)
    M = np.asarray(M, dtype=np.float32)
    z = np.asarray(z, dtype=np.float32)

    B, H, _, _ = query.shape
    BH = B * H
    qf = query.reshape(BH, S, D)
    kf = key.reshape(BH, S, D)
    vf = value.reshape(BH, S, D)
    mf = M.reshape(BH, D, D)
    zf = z.reshape(BH, D, 1)

    nc = _get_graph()
    in_maps = []
    for i in range(N_CORES):
        sl = slice(i * HP, (i + 1) * HP)
        in_maps.append({
            "query": np.ascontiguousarray(qf[sl]),
            "key": np.ascontiguousarray(kf[sl]),
            "value": np.ascontiguousarray(vf[sl]),
            "M": np.ascontiguousarray(mf[sl]),
            "z": np.ascontiguousarray(zf[sl]),
        })

    res = run_bass_kernel_spmd(nc, in_maps, list(range(N_CORES)), trace=trace)
    out = np.concatenate([res.results[i]["out"] for i in range(N_CORES)], axis=0)
    mn = np.concatenate([res.results[i]["M_new"] for i in range(N_CORES)], axis=0)
    zn = np.concatenate([res.results[i]["z_new"] for i in range(N_CORES)], axis=0)
    ret = (out.reshape(B, H, S, D), mn.reshape(B, H, D, D), zn.reshape(B, H, D, 1))
    if trace:
        return ret, res
    return ret
